# revision 45
# baseline (speedup 1.0000x reference)
"""Trainium2 Bass kernel for nn_AttentionalCopula (sparse_attention).

Sharding: data-parallel over batch (B=8 -> 8 cores); per-head K/V MLP stacks
computed locally per core, weights replicated, no collectives.

Key structure (v2):
  - The attention gather indices (left/right/mid/i) are host-known, so the
    K/V MLPs are evaluated only on the ~1.3k DISTINCT (s, t) rows the
    attention actually reads (padded to a multiple of 128), not all 2048.
  - Gathered K/V slots are fetched with single dma_gather ops (one SWDGE
    descriptor-gen instruction per phase) instead of 13 per-j indirect DMAs.
  - V is written/gathered in two head-halves so the weighted-sum of the
    final layer starts before the last V chains finish.
  - Bias+ReLU work is split between the ACT and DVE engines; the final
    256->64 projections are staged to SBUF by ACT.

v3: fp8 DoubleRow matmuls. Key-side errors cancel through the softmax
(shift-invariance) so K chains run fully in fp8e4m3 (W1/W2/W3 DoubleRow,
fp8 h1/h2); V output error is linear in the weights so V keeps bf16 for
W2/W3 and uses fp8 DoubleRow only for the W1 stage. Weights are scaled
x16 host-side (fp8e4m3 min-normal is 2^-6; raw weights sit at sigma=0.05)
and the scale is folded back in the drain scale/bias.
"""

from contextlib import ExitStack

import ml_dtypes
import numpy as np

import concourse.bass as bass
from concourse.instruction_name_ordered_set import InstructionNameOrderedSet


def _oset(*names):
    s = InstructionNameOrderedSet()
    for n in names:
        if n is not None:
            s.add(n)
    return s
import concourse.mybir as mybir
import concourse.tile as tile
from concourse import bacc
from concourse import library_config
from concourse.bass_utils import run_bass_kernel_spmd
from concourse.masks import make_identity

# problem constants (hardcoded per harness contract)
B, S, T, D = 8, 8, 256, 256
H, DK = 8, 64
HD = H * DK            # 512
L = 2
F = 256
R = 512
M = 128
EPS = 1e-5
NCORES = 8
P = 128
NSLOT = 2048           # 16 (s, n) pairs x 128 m

F32 = mybir.dt.float32
F32R = mybir.dt.float32r
BF16 = mybir.dt.bfloat16
I16 = mybir.dt.int16
F8 = mybir.dt.float8e4

NCHAIN = 2 * L * H     # 32 chains: c = (l*2 + kv)*8 + h
NKC = L * H            # 16 per-type chains (K or V), ck = l*8 + h
WSC = 16.0             # fp8 weight scale (lifts sigma=0.05 out of subnormals)
V_W1_FP8 = True        # V chains: W1 stage fp8 DoubleRow (False -> bf16)

# BCAST column layout (free-dim biases / LN params, replicated across partitions)
BC_DSB = 0                      # ds_b                  [512]
BC_B3V8 = 512                   # S * b3 of V chains, per l   [2*512]
BC_FFB2 = BC_B3V8 + L * HD      # ffb2 per l            [2*512]
BC_DEB3 = BC_FFB2 + L * HD      # deb3                  [512]
BC_LN1G = BC_DEB3 + R           # ln1_g per l           [2*512]
BC_LN1B = BC_LN1G + L * HD
BC_LN2G = BC_LN1B + L * HD
BC_LN2B = BC_LN2G + L * HD
BC_COLS = BC_LN2B + L * HD      # total

# PBIAS column layout (per-partition biases)
PB_B1 = 0                       # b1: 2 cols per chain (f-tile)    [64]
PB_B2 = PB_B1 + 2 * NCHAIN      # b2: 2 cols per chain             [64]
PB_FFB1 = PB_B2 + 2 * NCHAIN    # ffb1: 4 cols per l               [8]
PB_DEB1 = PB_FFB1 + 4 * L       # deb1: 2 cols                     [2]
PB_DEB2 = PB_DEB1 + 2           # deb2: 2 cols                     [2]
PB_LN2GT = PB_DEB2 + 2          # ln2_g[1] transposed, 4 kt cols   [4]
PB_LN2BT = PB_LN2GT + 4         # ln2_b[1] transposed              [4]
PB_COLS = PB_LN2BT + 4


def _tile_widths(nrowu):
    w, rem = [], nrowu
    while rem > 0:
        w.append(min(512, rem))
        rem -= w[-1]
    return w


def _emit(nc, tc, tensors, nrowu):
    XT8, XTC, PREDT, DSW, W1Q, W1B, W1C, W2Q, W2B, W3Q, W3B, \
        FFW, DEW1, DEW2, DEW3, \
        PBIAS, BCAST, IDXG, OUT, KD0, KD1, VL0, VH0, VL1A, VL1B, VH1 = tensors
    KD = [KD0, KD1]
    widths = _tile_widths(nrowu)
    NRT = nrowu // P        # stag row-blocks
    scale = DK ** (-0.5)

    with ExitStack() as ctx:
        cp = ctx.enter_context(tc.tile_pool(name="const", bufs=1))
        wp = ctx.enter_context(tc.tile_pool(name="w", bufs=4))
        hp = ctx.enter_context(tc.tile_pool(name="h", bufs=3))
        sp = ctx.enter_context(tc.tile_pool(name="stag", bufs=1))
        gp = ctx.enter_context(tc.tile_pool(name="gath", bufs=1))
        gp2 = ctx.enter_context(tc.tile_pool(name="gath2", bufs=2))
        ap = ctx.enter_context(tc.tile_pool(name="attn", bufs=1))
        ppd = ctx.enter_context(tc.tile_pool(name="psd", bufs=2, space="PSUM"))
        pp = ctx.enter_context(tc.tile_pool(name="ps", bufs=3, space="PSUM"))
        pa = ctx.enter_context(tc.tile_pool(name="psa", bufs=1, space="PSUM"))

        # ---- identity / eps before the gpsimd library switch ----
        ident = cp.tile([P, P], F32)
        make_identity(nc, ident[:])
        epst = cp.tile([P, 1], F32)
        nc.vector.memset(epst[:], EPS)
        nc.gpsimd.load_library(library_config.mlp)

        # ---- resident loads (chain-0 critical path first) ----
        # sync: first-chain weights come first (emitted by chain 0 below);
        # scalar: pbias then xt tile 0 then the rest.
        # 256 input features as 2 fp8 K-chunks (DoubleRow moving operand);
        # u + the constant-ones bias row ride in a separate bf16 chunk.
        xt8 = cp.tile([P, 2, nrowu], F8)
        nc.scalar.dma_start(xt8[:, :, 0:512], XT8.ap()[:, :, 0:512])
        nc.scalar.dma_start(xt8[:, :, 512:nrowu], XT8.ap()[:, :, 512:nrowu])
        xtc = cp.tile([P, nrowu], BF16)
        nc.scalar.dma_start(xtc[:], XTC.ap())
        pbias = cp.tile([P, PB_COLS], F32)
        nc.scalar.dma_start(pbias[:], PBIAS.ap())
        idxg = cp.tile([P, P], I16)
        nc.gpsimd.dma_start(idxg[:], IDXG.ap())
        bcast = cp.tile([P, BC_COLS], BF16)
        dew1 = cp.tile([P, 4, F], BF16)
        dew2 = cp.tile([P, 2, F], BF16)
        dew3 = cp.tile([P, 2, R], BF16)
        ffw = cp.tile([P, L, 2, 4, HD], BF16)   # [l][ff1/ff2][kt][hd]
        predt = cp.tile([P, 2, M], F32R)
        dsw = cp.tile([P, 2, HD], F32R)

        def ds_project():
            ps = pa.tile([P, HD], F32, tag="pa")
            nc.tensor.matmul(ps[:], predt[:, 0, :], dsw[:, 0, :], start=True, stop=False)
            nc.tensor.matmul(ps[:], predt[:, 1, :], dsw[:, 1, :], start=False, stop=True)
            av0 = ap.tile([P, HD], F32, tag="av")
            nc.vector.tensor_tensor(out=av0[:], in0=ps[:],
                                    in1=bcast[:, BC_DSB:BC_DSB + HD],
                                    op=mybir.AluOpType.add)
            return av0

        DR = mybir.MatmulPerfMode.DoubleRow

        def chain(l, kv, h, stag, hcol, act_drains=False):
            """One (l, kv, h) MLP chain over the unique rows; output columns
            [hcol*DK, (hcol+1)*DK) of stag.  Returns per-engine anchor
            instruction names used to pin phase math behind chain work."""
            c = (l * 2 + kv) * 8 + h
            ck = l * 8 + h
            isk = kv == 0
            w1fp8 = isk or V_W1_FP8
            if w1fp8:
                w1 = wp.tile([P, 2, F], F8, tag="w1")
                nc.sync.dma_start(w1[:], W1Q.ap()[c])
            else:
                w1 = wp.tile([P, 2, F], BF16, tag="w1b")
                nc.sync.dma_start(w1[:], W1B.ap()[ck])
            if not isk:
                w1c = wp.tile([P, F], BF16, tag="w1c")
                nc.sync.dma_start(w1c[:], W1C.ap()[c])
            if isk:
                w2 = wp.tile([P, 2, F], F8, tag="w2q")
                nc.sync.dma_start(w2[:], W2Q.ap()[ck])
                w3 = wp.tile([P, 2, DK], F8, tag="w3q")
                nc.sync.dma_start(w3[:], W3Q.ap()[ck])
            else:
                w2 = wp.tile([P, 2, F], BF16, tag="w2b")
                nc.sync.dma_start(w2[:], W2B.ap()[ck])
                w3 = wp.tile([P, 2, DK], BF16, tag="w3b")
                nc.sync.dma_start(w3[:], W3B.ap()[ck])

            hdt = F8 if isk else BF16
            anchors = {}
            col = 0
            for tt, w in enumerate(widths):
                cols = slice(col, col + w)
                h1 = hp.tile([P, 2, 512], hdt, tag="h1q" if isk else "h")
                # b1 rides in the bf16 chunk via the constant-ones input row;
                # ps1 carries 16x scale (fp8 weights are x16), undone in the
                # drain's activation scale.
                ps1 = ppd.tile([P, 2, 512], F32, tag="psd")
                for ft in range(2):
                    fsl = slice(ft * P, (ft + 1) * P)
                    if isk:
                        nc.tensor.matmul(ps1[:, ft, 0:w], w1[:, :, fsl],
                                         xt8[:, :, cols], start=True,
                                         stop=True, perf_mode=DR)
                        continue
                    if w1fp8:
                        nc.tensor.matmul(ps1[:, ft, 0:w], w1[:, :, fsl],
                                         xt8[:, :, cols], start=True,
                                         stop=False, perf_mode=DR)
                    else:
                        nc.tensor.matmul(ps1[:, ft, 0:w], w1[:, 0, fsl],
                                         xt8[:, 0, cols], start=True, stop=False)
                        nc.tensor.matmul(ps1[:, ft, 0:w], w1[:, 1, fsl],
                                         xt8[:, 1, cols], start=False, stop=False)
                    nc.tensor.matmul(ps1[:, ft, 0:w], w1c[:, fsl],
                                     xtc[:, cols], start=False, stop=True)
                if isk and tt == 2 and l == 0:
                    # drain-load balance: one K h1 tile per chain goes to DVE
                    # ((ps1/16) max 0 == relu(ps1)/16)
                    vi = nc.vector.tensor_scalar(
                        out=h1[:, :, 0:w], in0=ps1[:, :, 0:w],
                        scalar1=1.0 / WSC, scalar2=0.0,
                        op0=mybir.AluOpType.mult, op1=mybir.AluOpType.max)
                    anchors["dve"] = vi.ins.name
                else:
                    ai = nc.scalar.activation(
                        out=h1[:, :, 0:w], in_=ps1[:, :, 0:w],
                        func=mybir.ActivationFunctionType.Relu, scale=1.0 / WSC)
                    anchors["act"] = ai.ins.name
                h2 = hp.tile([P, 2, 512], hdt, tag="h2q" if isk else "h2")
                for gt in range(2):
                    gsl = slice(gt * P, (gt + 1) * P)
                    ps2 = pp.tile([P, 512], F32, tag="ps")
                    if isk:
                        nc.tensor.matmul(ps2[:, 0:w], w2[:, :, gsl],
                                         h1[:, :, 0:w], start=True, stop=True,
                                         perf_mode=DR)
                    else:
                        nc.tensor.matmul(ps2[:, 0:w], w2[:, 0, gsl],
                                         h1[:, 0, 0:w], start=True, stop=False)
                        nc.tensor.matmul(ps2[:, 0:w], w2[:, 1, gsl],
                                         h1[:, 1, 0:w], start=False, stop=True)
                    bc = PB_B2 + 2 * c + gt
                    if gt == 0 and not act_drains:
                        vi = nc.vector.tensor_scalar(
                            out=h2[:, gt, 0:w], in0=ps2[:, 0:w],
                            scalar1=pbias[:, bc:bc + 1], scalar2=0.0,
                            op0=mybir.AluOpType.add, op1=mybir.AluOpType.max)
                        anchors["dve"] = vi.ins.name
                    else:
                        ai = nc.scalar.activation(
                            out=h2[:, gt, 0:w], in_=ps2[:, 0:w],
                            func=mybir.ActivationFunctionType.Relu,
                            bias=pbias[:, bc:bc + 1])
                        anchors["act"] = ai.ins.name
                nrti = w // P
                ps3 = pp.tile([P, 512], F32, tag="ps")
                for rti in range(nrti):
                    dsl = slice(rti * DK, (rti + 1) * DK)
                    rsl = slice(rti * P, (rti + 1) * P)
                    if isk:
                        pi = nc.tensor.matmul(ps3[:, dsl], h2[:, :, rsl],
                                              w3[:, :, :], start=True,
                                              stop=True, perf_mode=DR)
                    else:
                        nc.tensor.matmul(ps3[:, dsl], h2[:, 0, rsl], w3[:, 0, :],
                                         start=True, stop=False)
                        pi = nc.tensor.matmul(ps3[:, dsl], h2[:, 1, rsl],
                                              w3[:, 1, :], start=False, stop=True)
                    anchors["pe"] = pi.ins.name
                sdst = stag[:, tt * 4:tt * 4 + nrti, hcol * DK:(hcol + 1) * DK]
                ssrc = ps3[:, 0:nrti * DK].rearrange("p (r d) -> p r d", d=DK)
                if isk:
                    nc.vector.tensor_scalar(
                        out=sdst, in0=ssrc,
                        scalar1=1.0 / (WSC * WSC), scalar2=None,
                        op0=mybir.AluOpType.mult,
                        op1=mybir.AluOpType.bypass)
                elif l == 0:
                    nc.vector.tensor_copy(sdst, ssrc)
                else:
                    nc.scalar.activation(
                        out=sdst, in_=ssrc,
                        func=mybir.ActivationFunctionType.Copy)
                col += w
            return anchors

        def ln_apply(src, mv, gcol, bcol, out, gb=True):
            rstd = ap.tile([P, 1], F32, tag="lnrstd")
            nc.scalar.activation(out=rstd[:], in_=mv[:, 1:2],
                                 func=mybir.ActivationFunctionType.Sqrt,
                                 bias=epst[:, 0:1])
            nc.vector.reciprocal(rstd[:], rstd[:])
            nc.vector.tensor_scalar(out=out[:], in0=src[:], scalar1=mv[:, 0:1],
                                    scalar2=rstd[:, 0:1],
                                    op0=mybir.AluOpType.subtract,
                                    op1=mybir.AluOpType.mult)
            if gb:
                nc.vector.tensor_tensor(out=out[:], in0=out[:],
                                        in1=bcast[:, gcol:gcol + HD],
                                        op=mybir.AluOpType.mult)
                nc.vector.tensor_tensor(out=out[:], in0=out[:],
                                        in1=bcast[:, bcol:bcol + HD],
                                        op=mybir.AluOpType.add)

        def layer_norm(src, gcol, bcol, out, gb=True):
            stats = ap.tile([P, 6], F32, tag="lnstat")
            nc.vector.bn_stats(stats[:], src[:])
            mv = ap.tile([P, 2], F32, tag="lnmv")
            nc.vector.bn_aggr(mv[:], stats[:])
            ln_apply(src, mv, gcol, bcol, out, gb=gb)

        def transpose_chunk(src, dst, kt, panch=None, pool=None, scale=1.0,
                            bias=0.0):
            if pool is not None:
                tpt = pool.tile([P, 512], F32, tag="ps", name="tpt")
                tpa = tpt[:, 0:P]
            else:
                tpt = pa.tile([P, P], F32, tag="pa", name="tpt")
                tpa = tpt[:]
            ti = nc.tensor.transpose(tpa, src[:, kt * P:(kt + 1) * P], ident[:])
            if panch is not None:
                ti.ins.add_sync_dependencies_from(_oset(panch))
            func = (mybir.ActivationFunctionType.Copy
                    if isinstance(bias, float)
                    else mybir.ActivationFunctionType.Identity)
            nc.scalar.activation(out=dst[:, kt, :], in_=tpa,
                                 func=func, scale=scale, bias=bias)

        def transpose_to(src, dst, panch=None, pool=None):
            """src [128, 512] f32 row-major -> dst [128, 4, 128] bf16 feature-major."""
            for kt in range(4):
                transpose_chunk(src, dst, kt, panch if kt == 0 else None,
                                pool=pool)

        scr16 = cp.tile([P, 4, HD], BF16)    # phase1 scratch [128, 2048]

        def attn_phase1_math(kgall, av16, wall, sems, trigs, anch):
            """Logits from gathered K; sigmoid pair-weights into wall.
            `anch`: (dve_name, act_name) pins where the scheduler may place
            this work, so the in-order engine queues don't stall on the
            not-yet-landed gather."""
            logits = ap.tile([P, 16, 8], BF16, tag="logits")
            avb = av16[:, None, :].to_broadcast([P, 4, HD])
            for js in range(4):
                jsl = slice(js * 4, (js + 1) * 4)
                mi = nc.vector.tensor_tensor(
                    out=scr16[:], in0=kgall[:, jsl, :], in1=avb,
                    op=mybir.AluOpType.mult)
                mi.ins.add_sync_dependencies_from(_oset(trigs, anch[0]))
                mi._wait_ge(sems[js // 2], 16)
                with nc.allow_low_precision(reason="logits feed a sigmoid of "
                                            "their difference; bf16 is ample"):
                    nc.vector.tensor_reduce(
                        out=logits[:, jsl, :],
                        in_=scr16[:].rearrange("p a (h d) -> p (a h) d", d=DK),
                        axis=mybir.AxisListType.X, op=mybir.AluOpType.add)
            delta = ap.tile([P, 8, 8], F32, tag="delta")
            nc.vector.tensor_tensor(out=delta[:], in0=logits[:, 0:8, :],
                                    in1=logits[:, 8:16, :],
                                    op=mybir.AluOpType.subtract)
            dflat = delta[:].rearrange("p a b -> p (a b)")
            nc.scalar.activation(
                out=wall[:, 0:64], in_=dflat,
                func=mybir.ActivationFunctionType.Sigmoid,
                scale=scale).ins.add_sync_dependencies_from(_oset(anch[1]))
            nc.scalar.activation(out=wall[:, 64:128], in_=dflat,
                                 func=mybir.ActivationFunctionType.Sigmoid, scale=-scale)

        def attn_wsum_part(vg, wall, h0, nh, att, sems, trigs, danch, coff=0):
            """Weighted sum of heads [h0, h0+nh) of gathered V into att."""
            HP = nh * DK
            wv = wall[:].rearrange("p (j h) -> p j h", h=8)
            # stride-0 broadcast of the pair-weights costs DVE its packed
            # fast path, but keeps the 3.4us expansion copy off ACT (which
            # head-of-line blocks the chain drains behind it)
            scr2 = ap.tile([P, 16, HP], BF16, tag="scr2", name="scr2")
            for js in range(4):
                jsl = slice(js * 4, (js + 1) * 4)
                mi = nc.vector.tensor_tensor(
                    out=scr2[:, jsl, :].rearrange("p a (h d) -> p a h d", d=DK),
                    in0=vg[:, jsl, coff:coff + HP].rearrange(
                        "p a (h d) -> p a h d", d=DK),
                    in1=wv[:, jsl, h0:h0 + nh, None]
                        .to_broadcast([P, 4, nh, DK]),
                    op=mybir.AluOpType.mult)
                mi.ins.add_sync_dependencies_from(_oset(trigs, danch))
                mi._wait_ge(sems[js // 2], 16)
            t8 = ap.tile([P, 8, HP], BF16, tag="t8", name="t8")
            nc.vector.tensor_tensor(out=t8[:], in0=scr2[:, 0:8, :],
                                    in1=scr2[:, 8:16, :], op=mybir.AluOpType.add)
            nc.vector.tensor_tensor(out=scr2[:, 0:4, :], in0=t8[:, 0:4, :],
                                    in1=t8[:, 4:8, :], op=mybir.AluOpType.add)
            nc.vector.tensor_tensor(out=t8[:, 0:2, :], in0=scr2[:, 0:2, :],
                                    in1=scr2[:, 2:4, :], op=mybir.AluOpType.add)
            nc.vector.tensor_tensor(out=att[:, h0 * DK:h0 * DK + HP],
                                    in0=t8[:, 0, :], in1=t8[:, 1, :],
                                    op=mybir.AluOpType.add)

        def attn_tail(l, avp, att, panch=None, pre_mv=None):
            """att += avp; LN; FF; LN -> next av (fp32) + av16 (bf16).
            l=1 runs after all chains: borrow the idle chain-stage PSUM pool
            so the FF groups double-buffer instead of serializing on pa."""
            def ptile():
                if l == 0:
                    return pa.tile([P, 512], F32, tag="pa", name="pt_a")
                return pp.tile([P, 512], F32, tag="ps", name="pt_p")
            xn = ap.tile([P, HD], F32, tag="xn")
            if pre_mv is None:
                nc.vector.tensor_tensor(out=att[:], in0=att[:], in1=avp[:],
                                        op=mybir.AluOpType.add)
                layer_norm(att, BC_LN1G + l * HD, BC_LN1B + l * HD, xn)
            else:
                ln_apply(att, pre_mv, BC_LN1G + l * HD, BC_LN1B + l * HD, xn)
            xT = ap.tile([P, 4, P], BF16, tag="xT")
            ff1 = ap.tile([P, 4, P], BF16, tag="ff1")
            transpose_to(xn, xT, panch, pool=pp if l == 1 else None)
            for ft in range(4):
                psf = ptile()[:, 0:P]
                for kt in range(4):
                    nc.tensor.matmul(psf, ffw[:, l, 0, kt, ft * P:(ft + 1) * P],
                                     xT[:, kt, :], start=(kt == 0), stop=(kt == 3))
                bc = PB_FFB1 + 4 * l + ft
                nc.scalar.activation(
                    out=ff1[:, ft, :], in_=psf,
                    func=mybir.ActivationFunctionType.Relu,
                    bias=pbias[:, bc:bc + 1])
            ps2 = ptile()
            for kt in range(4):
                nc.tensor.matmul(ps2[:, 0:HD], ff1[:, kt, :], ffw[:, l, 1, kt, :],
                                 start=(kt == 0), stop=(kt == 3))
            ffx = ap.tile([P, HD], F32, tag="ffx")
            nc.vector.tensor_tensor(out=ffx[:], in0=ps2[:, 0:HD],
                                    in1=bcast[:, BC_FFB2 + l * HD:BC_FFB2 + (l + 1) * HD],
                                    op=mybir.AluOpType.add)
            nc.vector.tensor_tensor(out=ffx[:], in0=ffx[:], in1=xn[:],
                                    op=mybir.AluOpType.add)
            av_out = ap.tile([P, HD], F32, tag="av")
            layer_norm(ffx, BC_LN2G + l * HD, BC_LN2B + l * HD, av_out,
                       gb=True)
            return av_out

        # ---- gather machinery: prepare_only SWDGE descriptor-gen is hoisted
        # into idle Pool windows; triggers fire after the source DRAM writes.
        # Each 1024-descriptor chunk fills a whole SWDGE ring, so chunks of
        # one phase go on two queues; queues alternate prep/trigger rounds.
        def prep_pair(dst, src_t, qa, qb, elem, tag):
            preps = []
            for g, q in ((0, qa), (1, qb)):
                sem = nc.alloc_semaphore(f"g_{tag}{g}")
                nc.gpsimd.dma_gather(dst[:, g * 8:(g + 1) * 8, :],
                                     src_t.ap().rearrange("p rt hd -> (p rt) hd"),
                                     idxg[:, g * 64:(g + 1) * 64],
                                     NSLOT // 2, NSLOT // 2, elem,
                                     prepare_only=True, sem=sem, queue_num=q)
                preps.append(sem)
            return preps

        def trig(qa, qb, wname):
            # the deferred RAW edge (source write -> DMA fire) sits on the
            # trigger: declare it as an IR dep so Tile wires the semaphores.
            t0 = nc.gpsimd.trigger_dma(count=None, queue_num=qa)
            t0.ins.add_sync_dependencies_from(_oset(wname))
            t1 = nc.gpsimd.trigger_dma(count=None, queue_num=qb)
            t1.ins.add_sync_dependencies_from(_oset(wname))
            return [t0.ins.name, t1.ins.name]

        def stag_write(dram_t, stag, name, eng=None):
            # verbatim partition-major write: one contiguous descriptor per
            # partition (vs 1.4k row-scattered ones); the gather indices are
            # remapped host-side to match
            inst = (eng or nc.scalar).dma_start(dram_t.ap(), stag[:])
            return inst.ins.name

        HH = HD // 2
        NA1 = 4 * DK            # layer-1 splits: 4 + 2 + 2 heads, so the
        NB1 = 2 * DK            # gathers fire after chains 4/6/8 and only the
        NH1 = 2 * DK            # last 2-head gather sits in the tail

        def cast16(src, aanch=None):
            dst = ap.tile([P, HD], BF16, tag="av16", name="av16")
            ci = nc.scalar.activation(out=dst[:], in_=src[:],
                                      func=mybir.ActivationFunctionType.Copy)
            if aanch is not None:
                ci.ins.add_sync_dependencies_from(_oset(aanch))
            return dst

        def fold_b3v(l, av):
            avp = ap.tile([P, HD], F32, tag="avp", name="avp")
            nc.vector.tensor_tensor(
                out=avp[:], in0=av[:],
                in1=bcast[:, BC_B3V8 + l * HD:BC_B3V8 + (l + 1) * HD],
                op=mybir.AluOpType.add)
            return avp

        # gather destination tiles (allocated up-front: preps reference them)
        kg0 = gp.tile([P, 16, HD], BF16, tag="kgall", name="kg0")
        kg1 = gp.tile([P, 16, HD], BF16, tag="kgall", name="kg1")
        vgl0 = gp2.tile([P, 16, HH], BF16, tag="vgl", name="vgl0")
        vgl1a = gp2.tile([P, 16, NA1], BF16, tag="vgl", name="vgl1a")
        vgh0 = gp.tile([P, 16, HH], BF16, tag="vgh", name="vgh0")
        vgl1b = gp.tile([P, 16, NB1], BF16, tag="vgh", name="vgl1b")
        vgh1 = gp.tile([P, 16, NH1], BF16, tag="vgh1", name="vgh1")

        # round-1 preps: descriptor-gen runs during the K0 chains
        sem_k0 = prep_pair(kg0, KD[0], 0, 1, HD, "k0")
        sem_vl0 = prep_pair(vgl0, VL0, 2, 3, HH, "vl0")


        wall0 = ap.tile([P, 128], F32, tag="wall")
        wall1 = ap.tile([P, 128], F32, tag="wall1")

        # ---- l=0: K chains ----
        kstag = sp.tile([P, NRT, HD], BF16, tag="kstag", name="kstag0")
        for h in range(4):
            chain(0, 0, h, kstag, h)
        # bulky replicated params issue behind the first chains' weight
        # loads (sync queue is HWDGE: no SWDGE-ring interaction with the
        # untriggered gather preps parked on the Pool queues)
        nc.sync.dma_start(bcast[:], BCAST.ap())
        nc.sync.dma_start(predt[:], PREDT.ap())
        nc.sync.dma_start(dsw[:], DSW.ap())
        nc.sync.dma_start(ffw[:], FFW.ap())
        nc.sync.dma_start(dew1[:], DEW1.ap())
        nc.sync.dma_start(dew2[:], DEW2.ap())
        nc.sync.dma_start(dew3[:], DEW3.ap())
        for h in range(4, H):
            chain(0, 0, h, kstag, h)
        av = ds_project()
        av16 = cast16(av)
        wk0 = stag_write(KD[0], kstag, "kd0")
        trig(0, 1, wk0)                             # K0 gather fires
        sem_vh0 = prep_pair(vgh0, VH0, 0, 1, HH, "vh0")    # round-2 preps

        # ---- l=0: V chains (lo head-half, then hi) ----
        vstag_lo = sp.tile([P, NRT, HH], BF16, tag="vstag_lo", name="vlo0")
        vstag_hi = sp.tile([P, NRT, HH], BF16, tag="vstag_hi", name="vhi0")
        a_v0 = [chain(0, 1, h, vstag_lo, h) for h in range(3)]
        avp0 = fold_b3v(0, av)
        # phase1(l0): emitted mid-V0 (engine queues are in-order, so emission
        # position IS queue position); anchored so the gather has landed when
        # the DVE/ACT queues reach it
        attn_phase1_math(kg0, av16, wall0, sem_k0, wk0,
                         (a_v0[1]["dve"], a_v0[2]["act"]))
        a_v0.append(chain(0, 1, 3, vstag_lo, 3))
        wvl0 = stag_write(VL0, vstag_lo, "vl0")
        trig(2, 3, wvl0)                            # VL0 fires
        sem_k1 = prep_pair(kg1, KD[1], 2, 3, HD, "k1")        # round-2 preps
        a_v0 += [chain(0, 1, h, vstag_hi, h - 4) for h in range(4, 8)]
        wvh0 = stag_write(VH0, vstag_hi, "vh0")
        trig(0, 1, wvh0)                            # VH0 fires
        sem_vl1a = prep_pair(vgl1a, VL1A, 0, 1, NA1, "vl1a")  # round-3 preps

        # ---- l=1: K chains; l=0 attention consumption interleaved ----
        att0 = ap.tile([P, HD], F32, tag="att", name="att0")
        kstag1 = sp.tile([P, NRT, HD], BF16, tag="kstag1", name="kstag1")
        a_k1 = [chain(1, 0, h, kstag1, h) for h in range(2)]
        attn_wsum_part(vgl0, wall0, 0, 4, att0, sem_vl0, wvl0, a_k1[0]["dve"])
        a_k1.append(chain(1, 0, 2, kstag1, 2))
        attn_wsum_part(vgh0, wall0, 4, 4, att0, sem_vh0, wvh0, a_k1[1]["dve"])
        a_k1 += [chain(1, 0, h, kstag1, h) for h in range(3, 6)]
        av = attn_tail(0, avp0, att0, panch=a_k1[4]["pe"])
        av16 = cast16(av, aanch=a_k1[5]["act"])
        a_k1 += [chain(1, 0, h, kstag1, h) for h in range(6, 8)]
        wk1 = stag_write(KD[1], kstag1, "kd1")
        trig(2, 3, wk1)                             # K1 fires
        sem_vl1b = prep_pair(vgl1b, VL1B, 2, 3, NB1, "vl1b")  # round-3 preps

        # ---- l=1: V chains; l=1 attention interleaved so the tail is short ----
        vstag_a1 = sp.tile([P, NRT, NA1], BF16, tag="vstag_a1", name="va1")
        vstag_b1 = sp.tile([P, NRT, NB1], BF16, tag="vstag_b1", name="vb1")
        vstag_hi1 = sp.tile([P, NRT, NH1], BF16, tag="vstag_hi1", name="vhi1")
        a_v1 = [chain(1, 1, h, vstag_a1, h) for h in range(3)]
        avp1 = fold_b3v(1, av)
        attn_phase1_math(kg1, av16, wall1, sem_k1, wk1,
                         (a_v1[1]["dve"], a_v1[2]["act"]))
        a_v1.append(chain(1, 1, 3, vstag_a1, 3))
        wvla = stag_write(VL1A, vstag_a1, "vl1a")
        trig(0, 1, wvla)                            # VL1A (heads 0-3) fires
        sem_vh1 = prep_pair(vgh1, VH1, 0, 1, NH1, "vh1")      # round-4 prep
        a_v1.append(chain(1, 1, 4, vstag_b1, 0))
        a_v1.append(chain(1, 1, 5, vstag_b1, 1))
        wvlb = stag_write(VL1B, vstag_b1, "vl1b")
        trig(2, 3, wvlb)                            # VL1B (heads 4-5) fires
        att1 = ap.tile([P, HD], F32, tag="att", name="att1")
        a_v1.append(chain(1, 1, 6, vstag_hi1, 0, act_drains=True))
        attn_wsum_part(vgl1a, wall1, 0, 4, att1, sem_vl1a,
                       wvla, a_v1[5]["dve"])
        attn_wsum_part(vgl1b, wall1, 4, 2, att1, sem_vl1b,
                       wvlb, a_v1[5]["dve"])
        a_v1.append(chain(1, 1, 7, vstag_hi1, 1, act_drains=True))
        NLC = 6 * DK
        stats1 = ap.tile([P, 2, 6], F32, tag="lnstat1", name="stats1")
        nc.vector.tensor_tensor(out=att1[:, 0:NLC], in0=att1[:, 0:NLC],
                                in1=avp1[:, 0:NLC], op=mybir.AluOpType.add)
        nc.vector.bn_stats(stats1[:, 0, :], att1[:, 0:NLC])
        wvh1 = stag_write(VH1, vstag_hi1, "vh1", eng=nc.sync)
        trig(0, 1, wvh1)                            # VH1: only the transfer
        attn_wsum_part(vgh1, wall1, 6, 2, att1, sem_vh1, wvh1, a_v1[7]["act"])
        nc.vector.tensor_tensor(out=att1[:, NLC:HD], in0=att1[:, NLC:HD],
                                in1=avp1[:, NLC:HD], op=mybir.AluOpType.add)
        nc.vector.bn_stats(stats1[:, 1, :], att1[:, NLC:HD])
        mv1 = ap.tile([P, 2], F32, tag="lnmv")
        nc.vector.bn_aggr(mv1[:], stats1[:])
        av = attn_tail(1, avp1, att1, pre_mv=mv1)

        # ---- dist extractor (ln2 gamma/beta ride in the transpose copies) ----
        avT = ap.tile([P, 4, P], BF16, tag="avT")
        transpose_to(av, avT, pool=pp)
        h1 = ap.tile([P, 2, P], BF16, tag="deh1")
        for ft in range(2):
            psdt = pp.tile([P, 512], F32, tag="ps", name="psdt")
            psd = psdt[:, 0:P]
            for kt in range(4):
                nc.tensor.matmul(psd, dew1[:, kt, ft * P:(ft + 1) * P],
                                 avT[:, kt, :], start=(kt == 0), stop=(kt == 3))
            nc.scalar.activation(out=h1[:, ft, :], in_=psd,
                                 func=mybir.ActivationFunctionType.Relu,
                                 bias=pbias[:, PB_DEB1 + ft:PB_DEB1 + ft + 1])
        h2 = ap.tile([P, 2, P], BF16, tag="deh2")
        for ft in range(2):
            psdt = pp.tile([P, 512], F32, tag="ps", name="psdt")
            psd = psdt[:, 0:P]
            for kt in range(2):
                nc.tensor.matmul(psd, dew2[:, kt, ft * P:(ft + 1) * P],
                                 h1[:, kt, :], start=(kt == 0), stop=(kt == 1))
            nc.scalar.activation(out=h2[:, ft, :], in_=psd,
                                 func=mybir.ActivationFunctionType.Relu,
                                 bias=pbias[:, PB_DEB2 + ft:PB_DEB2 + ft + 1])
        pso = pp.tile([P, 512], F32, tag="ps", name="pso")
        for kt in range(2):
            nc.tensor.matmul(pso[:], h2[:, kt, :], dew3[:, kt, :],
                             start=(kt == 0), stop=(kt == 1))
        o = ap.tile([P, R], F32, tag="out")
        nc.vector.tensor_tensor(out=o[:], in0=pso[:],
                                in1=bcast[:, BC_DEB3:BC_DEB3 + R],
                                op=mybir.AluOpType.add)
        nc.sync.dma_start(OUT.ap(), o[:])


_BUILD_CACHE = {}


def _build(nrowu):
    if nrowu in _BUILD_CACHE:
        return _BUILD_CACHE[nrowu]
    nc = bacc.Bacc("TRN2", target_bir_lowering=False, debug=False,
                   num_swdge_queues=4)
    t = []
    t.append(nc.dram_tensor("XT8", [P, 2, nrowu], F8, kind="ExternalInput"))
    t.append(nc.dram_tensor("XTC", [P, nrowu], BF16, kind="ExternalInput"))
    t.append(nc.dram_tensor("PREDT", [P, 2, M], F32R, kind="ExternalInput"))
    t.append(nc.dram_tensor("DSW", [P, 2, HD], F32R, kind="ExternalInput"))
    t.append(nc.dram_tensor("W1Q", [NCHAIN, P, 2, F], F8, kind="ExternalInput"))
    t.append(nc.dram_tensor("W1B", [NKC, P, 2, F], BF16, kind="ExternalInput"))
    t.append(nc.dram_tensor("W1C", [NCHAIN, P, F], BF16, kind="ExternalInput"))
    t.append(nc.dram_tensor("W2Q", [NKC, P, 2, F], F8, kind="ExternalInput"))
    t.append(nc.dram_tensor("W2B", [NKC, P, 2, F], BF16, kind="ExternalInput"))
    t.append(nc.dram_tensor("W3Q", [NKC, P, 2, DK], F8, kind="ExternalInput"))
    t.append(nc.dram_tensor("W3B", [NKC, P, 2, DK], BF16, kind="ExternalInput"))
    t.append(nc.dram_tensor("FFW", [P, L, 2, 4, HD], BF16, kind="ExternalInput"))
    t.append(nc.dram_tensor("DEW1", [P, 4, F], BF16, kind="ExternalInput"))
    t.append(nc.dram_tensor("DEW2", [P, 2, F], BF16, kind="ExternalInput"))
    t.append(nc.dram_tensor("DEW3", [P, 2, R], BF16, kind="ExternalInput"))
    t.append(nc.dram_tensor("PBIAS", [P, PB_COLS], F32, kind="ExternalInput"))
    t.append(nc.dram_tensor("BCAST", [P, BC_COLS], BF16, kind="ExternalInput"))
    t.append(nc.dram_tensor("IDXG", [P, P], I16, kind="ExternalInput"))
    t.append(nc.dram_tensor("OUT", [M, R], F32, kind="ExternalOutput"))
    nrt = nrowu // P
    t.append(nc.dram_tensor("KD0", [P, nrt, HD], BF16))
    t.append(nc.dram_tensor("KD1", [P, nrt, HD], BF16))
    t.append(nc.dram_tensor("VL0", [P, nrt, HD // 2], BF16))
    t.append(nc.dram_tensor("VH0", [P, nrt, HD // 2], BF16))
    t.append(nc.dram_tensor("VL1A", [P, nrt, 4 * DK], BF16))
    t.append(nc.dram_tensor("VL1B", [P, nrt, 2 * DK], BF16))
    t.append(nc.dram_tensor("VH1", [P, nrt, 2 * DK], BF16))
    with tile.TileContext(nc) as tc:
        _emit(nc, tc, t, nrowu)
    nc.compile()
    _BUILD_CACHE[nrowu] = nc
    return nc


def _unique_map(i, left, right):
    """Sorted unique (s, t) rows + per-slot positions (j = n*8+s, m)."""
    pairs = set()
    for s in range(S):
        for m in range(M):
            pairs.add((s, int(left[m])))
            pairs.add((s, m if s < i else int(right[m])))
    rows = sorted(pairs)
    pos = {r: k for k, r in enumerate(rows)}
    nu = len(rows)
    nrowu = ((nu + P - 1) // P) * P
    s_list = np.array([r[0] for r in rows], np.int64)
    t_list = np.array([r[1] for r in rows], np.int64)
    idx16 = np.empty((M, 16), np.int64)
    for s in range(S):
        for m in range(M):
            idx16[m, 0 * 8 + s] = pos[(s, int(left[m]))]
            idx16[m, 1 * 8 + s] = pos[(s, m if s < i else int(right[m]))]
    # partition-major row numbering: logical row r (= rt*128 + p) lives at
    # DRAM row p*nrt + rt after the verbatim staging write
    nrt = nrowu // P
    idx16 = (idx16 % P) * nrt + (idx16 // P)
    # dma_gather idx packing: idx k lives at partition k%16, column k//16,
    # replicated across the eight 16-partition groups.
    flat = idx16.T.reshape(-1)                    # [2048], slot = j*128 + m
    w16 = flat.reshape(P, 16).T.astype(np.int16)  # [16, 128]
    idxg = np.tile(w16, (8, 1))                   # [128, 128]
    return s_list, t_list, idxg, nrowu


def _prep_shared(ins):
    """Pack weights/biases (identical across cores)."""
    f32 = np.float32
    bf16 = ml_dtypes.bfloat16
    e4 = ml_dtypes.float8_e4m3
    kW1, kW2, kW3 = ins["kW1"], ins["kW2"], ins["kW3"]
    vW1, vW2, vW3 = ins["vW1"], ins["vW2"], ins["vW3"]
    kb1, kb2 = ins["kb1"], ins["kb2"]
    vb1, vb2, vb3 = ins["vb1"], ins["vb2"], ins["vb3"]

    W1Q = np.zeros((NCHAIN, P, 2, F), e4)
    W1B = np.zeros((NKC, P, 2, F), bf16)
    W1C = np.zeros((NCHAIN, P, F), bf16)
    W2Q = np.zeros((NKC, P, 2, F), e4)
    W2B = np.zeros((NKC, P, 2, F), bf16)
    W3Q = np.zeros((NKC, P, 2, DK), e4)
    W3B = np.zeros((NKC, P, 2, DK), bf16)
    PB = np.zeros((P, PB_COLS), f32)
    BC = np.zeros((BC_COLS,), f32)

    for l in range(L):
        for kv in range(2):
            for h in range(H):
                c = (l * 2 + kv) * 8 + h
                ck = l * 8 + h
                w1, w2, w3 = (kW1, kW2, kW3) if kv == 0 else (vW1, vW2, vW3)
                b1, b2 = (kb1, kb2) if kv == 0 else (vb1, vb2)
                w1m = np.asarray(w1[l, h], f32)           # [257, F]
                w1s = (WSC * w1m[:D]).reshape(2, P, F).transpose(1, 0, 2)
                W1Q[c] = w1s.astype(e4)
                if kv == 1:
                    W1B[ck] = w1s.astype(bf16)
                w1cp = np.zeros((P, F), f32)
                w1cp[0] = WSC * w1m[D]                    # u weight
                w1cp[1] = WSC * np.asarray(b1[l, h], f32)  # b1 via ones row
                W1C[c] = w1cp.astype(bf16)
                w2m = np.asarray(w2[l, h], f32).reshape(2, P, F).transpose(1, 0, 2)
                w3m = np.asarray(w3[l, h], f32).reshape(2, P, DK).transpose(1, 0, 2)
                if kv == 0:
                    W2Q[ck] = (WSC * w2m).astype(e4)
                    W3Q[ck] = (WSC * w3m).astype(e4)
                    b2s = WSC
                else:
                    W2B[ck] = w2m.astype(bf16)
                    W3B[ck] = w3m.astype(bf16)
                    b2s = 1.0
                for ft in range(2):
                    PB[:, PB_B2 + 2 * c + ft] = b2s * np.asarray(
                        b2[l, h][ft * P:(ft + 1) * P], f32)

    DSW = np.asarray(ins["ds_W"], f32).reshape(2, P, HD).transpose(1, 0, 2).copy()
    BC[BC_DSB:BC_DSB + HD] = np.asarray(ins["ds_b"], f32)
    for l in range(L):
        # V-chain output bias folded through softmax: sum_(s,n) w*b3v = S*b3v
        BC[BC_B3V8 + l * HD:BC_B3V8 + (l + 1) * HD] = \
            S * np.asarray(vb3[l], f32).reshape(HD)

    FFW = np.empty((P, L, 2, 4, HD), bf16)
    for l in range(L):
        FFW[:, l, 0] = np.asarray(ins["ffW1"][l], f32).reshape(4, P, HD).transpose(1, 0, 2).astype(bf16)
        FFW[:, l, 1] = np.asarray(ins["ffW2"][l], f32).reshape(4, P, HD).transpose(1, 0, 2).astype(bf16)
        for ft in range(4):
            PB[:, PB_FFB1 + 4 * l + ft] = np.asarray(ins["ffb1"][l][ft * P:(ft + 1) * P], f32)
        BC[BC_FFB2 + l * HD:BC_FFB2 + (l + 1) * HD] = np.asarray(ins["ffb2"][l], f32)
        BC[BC_LN1G + l * HD:BC_LN1G + (l + 1) * HD] = np.asarray(ins["ln1_g"][l], f32)
        BC[BC_LN1B + l * HD:BC_LN1B + (l + 1) * HD] = np.asarray(ins["ln1_b"][l], f32)
        BC[BC_LN2G + l * HD:BC_LN2G + (l + 1) * HD] = np.asarray(ins["ln2_g"][l], f32)
        BC[BC_LN2B + l * HD:BC_LN2B + (l + 1) * HD] = np.asarray(ins["ln2_b"][l], f32)

    for kt in range(4):
        PB[:, PB_LN2GT + kt] = np.asarray(ins["ln2_g"][L - 1][kt * P:(kt + 1) * P], f32)
        PB[:, PB_LN2BT + kt] = np.asarray(ins["ln2_b"][L - 1][kt * P:(kt + 1) * P], f32)
    DEW1 = np.asarray(ins["deW1"], f32).reshape(4, P, F).transpose(1, 0, 2).astype(bf16)
    DEW2 = np.asarray(ins["deW2"], f32).reshape(2, P, F).transpose(1, 0, 2).astype(bf16)
    DEW3 = np.asarray(ins["deW3"], f32).reshape(2, P, R).transpose(1, 0, 2).astype(bf16)
    for ft in range(2):
        PB[:, PB_DEB1 + ft] = np.asarray(ins["deb1"][ft * P:(ft + 1) * P], f32)
        PB[:, PB_DEB2 + ft] = np.asarray(ins["deb2"][ft * P:(ft + 1) * P], f32)
    BC[BC_DEB3:BC_DEB3 + R] = np.asarray(ins["deb3"], f32)

    BCAST = np.broadcast_to(BC, (P, BC_COLS)).astype(ml_dtypes.bfloat16)

    return {
        "W1Q": W1Q, "W1B": W1B, "W1C": W1C, "W2Q": W2Q, "W2B": W2B,
        "W3Q": W3Q, "W3B": W3B, "DSW": DSW,
        "FFW": FFW, "DEW1": DEW1, "DEW2": DEW2, "DEW3": DEW3,
        "PBIAS": PB, "BCAST": BCAST,
    }


def make_in_maps(ins):
    shared = _prep_shared(ins)
    enc = np.asarray(ins["encoded"], np.float32)        # [B, S, T, D]
    tu = np.asarray(ins["true_u"], np.float32)          # [B, S, T]
    mid = np.asarray(ins["mid_idx"], np.int64)
    left = np.asarray(ins["left_idx"], np.int64)
    right = np.asarray(ins["right_idx"], np.int64)
    i = int(ins["i"])

    s_list, t_list, idxg, nrowu = _unique_map(i, left, right)
    nu = len(s_list)
    shared["IDXG"] = idxg

    in_maps = []
    for b in range(B):
        xe = np.zeros((D, nrowu), np.float32)         # fp8 feature chunks
        xe[:, :nu] = enc[b][s_list, t_list].T
        xt8 = xe.reshape(2, P, nrowu).transpose(1, 0, 2).astype(
            ml_dtypes.float8_e4m3)
        xc = np.zeros((P, nrowu), np.float32)         # bf16 chunk: u + ones
        xc[0, :nu] = tu[b][s_list, t_list]
        xc[1, :] = 1.0                                # bias row
        xtc = xc.astype(ml_dtypes.bfloat16)
        pred = enc[b, i][mid]                           # [M, D]
        predt = pred.T.reshape(2, P, M).transpose(1, 0, 2).copy()
        m = dict(shared)
        m["XT8"] = xt8
        m["XTC"] = xtc
        m["PREDT"] = predt
        in_maps.append(m)
    return in_maps, nrowu


def kernel(**inputs):
    ins = {k: np.asarray(v) for k, v in inputs.items()}
    in_maps, nrowu = make_in_maps(ins)
    nc = _build(nrowu)
    res = run_bass_kernel_spmd(nc, in_maps, core_ids=list(range(NCORES)))
    out = np.stack([res.results[c]["OUT"] for c in range(NCORES)])
    return out.astype(np.float32)                       # [B, M, R]

